# revision 58
# baseline (speedup 1.0000x reference)
"""Trainium2 Bass kernel for CrossAttentionFusion.

Reference computation (per batch element b, torch Linear convention):
    V = Xkv @ Wv.T + bv            [Skv, D]
    K = Xkv @ Wk.T + bk            [Skv, D]
    Q = Xq  @ Wq.T + bq            [Sq, D]
    E = Q @ K.T / sqrt(128)        [Sq, Skv]
    A = softmax(E, axis=-1)
    F = A @ V                      [Sq, D]
    O = F @ Wd.T + bd              [Sq, D]

Sharding: data-parallel over batch, B=32 across 8 cores (4 per core).

Engine-balanced device strategy (ACT-exp is the binding roofline; GPSIMD
may not touch PSUM, so every PSUM reader is PE/ACT/DVE):
  - inputs loaded f32, cast to bf16 on GPSIMD, transposed to feature-major
    [D, S] by the DMA XBAR (dma_start_transpose) - zero PE cost
  - all matmul operands bf16 (1 cycle/row at any free size); PSUM stays f32
  - all params arrive in ONE packed DMA (wb) issued from the ACT DGE; the
    global HWDGE token serializes DMA issues ~625ns each, so DMA instruction
    count is minimized and ordered: input loads first, kv before q
  - QT = Wq @ XqT (+bq), KT = Wk @ XkvT (+bk)   [D, S]   (bias on DVE)
  - V = Xkv @ Wv.T (+bv)                        [Skv, D] (bias on DVE)
  - attention per q-chunk (512 wide), kv supertiles of 2x128:
       E^T pair   -> one [128, 1024] 2-bank PSUM tile    (PE)
       A = exp(E^T/sqrt(128)) 1024-wide -> bf16 SBUF     (ACT, bottleneck)
       F^T       += V_k-matmul A_half                    (PE)
       row-sums via bf16 pairwise add-tree (L1 on GPSIMD, rest on DVE)
    softmax denominators: tiny accumulating matmuls l3^T @ ones -> [128q, 1]
    per subtile, then DVE reciprocal; normalization folded into the
    O-projection (per-partition scale), deferred into the next chunk's
    pipeline; next batch's KT/QT/V projections are likewise deferred into
    this batch's chunk slots so batch boundaries have no PE lump.
  - O tile = FT-matmul WdT, then (*recipS + bd) on DVE, DMA to HBM.

softmax max-subtraction is skipped: E ~ N(0,1) for these inputs, exp() is
well within fp32 range; matches jax softmax to fp rounding.
"""

import numpy as np

B_TOTAL = 32
N_CORES = 8
B_PER_CORE = B_TOTAL // N_CORES
SQ = 2048
SKV = 2048
D = 128
P = 128
QCHUNK = 512
NT_Q = SQ // P        # 16 q tiles per batch
NT_KV = SKV // P      # 16 kv tiles per batch
NC_Q = SQ // QCHUNK   # 4 q chunks per batch
QSUB = QCHUNK // P    # 4 q subtiles per chunk
NST = NT_KV // 2      # 8 kv supertiles (2 tiles each) per chunk
LA = 2                # supertile lookahead (AV/tree trail E/exp by LA)
SCALE = 1.0 / np.sqrt(128.0)

_PROGRAM_CACHE = {}


def build_program(n_batch=B_PER_CORE, n_iters=1):
    import concourse.mybir as mybir
    import concourse.tile as tile
    from concourse import bacc
    from concourse.masks import make_identity
    from contextlib import ExitStack

    f32 = mybir.dt.float32
    bf16 = mybir.dt.bfloat16
    AF = mybir.ActivationFunctionType
    Alu = mybir.AluOpType

    nc = bacc.Bacc("TRN2", target_bir_lowering=False, debug=False)

    xq_d = nc.dram_tensor("xq", [n_batch, SQ, D], f32, kind="ExternalInput")
    xkv_d = nc.dram_tensor("xkv", [n_batch, SKV, D], f32, kind="ExternalInput")
    # all params packed host-side into one tensor -> one DMA:
    # [:, 0:512] = wk|wq|wv|wd natural [d_out, d_in], [:, 512:516] = bk|bq|bv|bd
    wb_d = nc.dram_tensor("wb", [D, 4 * D + 4], f32, kind="ExternalInput")
    out_d = nc.dram_tensor("out", [n_batch, SQ, D], f32, kind="ExternalOutput")

    with tile.TileContext(nc) as tc, ExitStack() as ctx:
        const = ctx.enter_context(tc.tile_pool(name="const", bufs=1))
        xin_pool = ctx.enter_context(tc.tile_pool(name="xin", bufs=4))
        xbf_pool = ctx.enter_context(tc.tile_pool(name="xbf", bufs=4))
        xt_pool = ctx.enter_context(tc.tile_pool(name="xt", bufs=4))
        qkv_pool = ctx.enter_context(tc.tile_pool(name="qkv", bufs=6))
        ft_pool = ctx.enter_context(tc.tile_pool(name="ft", bufs=2))
        a_pool = ctx.enter_context(tc.tile_pool(name="a", bufs=4))
        tr_pool = ctx.enter_context(tc.tile_pool(name="tr", bufs=16))
        r_pool = ctx.enter_context(tc.tile_pool(name="r", bufs=2))
        o_pool = ctx.enter_context(tc.tile_pool(name="o", bufs=2))
        # PSUM: 3x2-bank E supertiles + 1 F bank + 1 shared misc bank = 8
        e_psum = ctx.enter_context(tc.tile_pool(name="e_psum", bufs=3, space="PSUM"))
        f_psum = ctx.enter_context(tc.tile_pool(name="f_psum", bufs=1, space="PSUM"))
        m_psum = ctx.enter_context(tc.tile_pool(name="m_psum", bufs=1, space="PSUM"))

        # ---- constants: ONE packed DMA (issued from the ACT DGE so it does
        # not block the input loads on the SP queue), everything else derived
        # on-chip ----
        ident = const.tile([P, P], f32)
        make_identity(nc, ident)
        ones_col = const.tile([P, 1], bf16)
        nc.vector.memset(ones_col, 1.0)
        ones_row = const.tile([1, P], bf16)
        nc.vector.memset(ones_row, 1.0)

        wb = const.tile([P, 4 * D + 4], f32, tag="wb")
        nc.scalar.dma_start(wb[:], wb_d[:, :])

        # per-partition bias columns (d_out on partitions)
        bcol = {"bk": wb[:, 4 * P:4 * P + 1], "bq": wb[:, 4 * P + 1:4 * P + 2]}
        crow = {}
        wT = {}
        bdbc = const.tile([P, P], f32, tag="bdbc")
        W_IDX = {"wk": 0, "wq": 1, "wv": 2, "wd": 3}

        # early consts in ONE psum tile (pool-tile rotations serialize via
        # semaphore round-trips; fewer allocations = faster first E-matmul):
        # wkT | wqT | wvT transposes + the bv row.  Copies run on the idle
        # GPSIMD so the DVE chain never gates the first projections.
        ec_ps = e_psum.tile([P, 2 * QCHUNK], f32, tag="e")
        for i, n in enumerate(("wk", "wq", "wv")):
            nc.tensor.transpose(ec_ps[:, i * P:(i + 1) * P],
                                wb[:, W_IDX[n] * P:(W_IDX[n] + 1) * P],
                                ident[:])
        for i, n in enumerate(("wk", "wq", "wv")):
            wt = const.tile([P, P], bf16, tag=f"{n}T")
            nc.vector.tensor_copy(wt[:], ec_ps[:, i * P:(i + 1) * P])
            wT[n] = wt

        def emit_consts_late():
            ps = m_psum.tile([P, 2 * P], f32, tag="m")
            nc.tensor.transpose(ps[:, :P], wb[:, 3 * P:4 * P], ident[:])
            nc.tensor.matmul(ps[0:1, P:2 * P],
                             lhsT=wb[:, 4 * P + 3:4 * P + 4], rhs=ident[:],
                             start=True, stop=True)
            wt = const.tile([P, P], bf16, tag="wdT")
            nc.vector.tensor_copy(wt[:], ps[:, :P])
            wT["wd"] = wt
            brow_bd = const.tile([1, P], bf16, tag="bdr")
            nc.vector.tensor_copy(brow_bd[:], ps[0:1, P:2 * P])
            ps2 = m_psum.tile([P, P], f32, tag="m")
            nc.tensor.matmul(ps2[:, :P], lhsT=ones_row[:], rhs=brow_bd[:],
                             start=True, stop=True)
            nc.vector.tensor_copy(bdbc[:], ps2[:, :P])

        # deferred O-projection state: (FT, recipS, batch, chunk)
        pending_oproj = []

        def emit_oproj(FT, recipS, b, c, split_dma=False):
            o_ps = m_psum.tile([P, QCHUNK], f32, tag="m")
            for j in range(QSUB):
                t = c * QSUB + j
                nc.tensor.matmul(o_ps[:, j * P:(j + 1) * P],
                                 lhsT=FT[:, t * P:(t + 1) * P],
                                 rhs=wT["wd"][:], start=True, stop=True)
            o_sb = o_pool.tile([P, QSUB, P], f32, tag="o")
            for j in range(QSUB):
                t = c * QSUB + j
                nc.vector.scalar_tensor_tensor(
                    o_sb[:, j, :], o_ps[:, j * P:(j + 1) * P],
                    recipS[:, t:t + 1], bdbc[:],
                    op0=Alu.mult, op1=Alu.add)
            out_r = out_d[b].rearrange("(t p) d -> p t d", p=P)
            if split_dma:
                # drain path: don't make the final DMA wait on all subtiles
                h = QSUB // 2
                nc.sync.dma_start(out_r[:, c * QSUB:c * QSUB + h, :],
                                  o_sb[:, :h, :])
                nc.sync.dma_start(out_r[:, c * QSUB + h:(c + 1) * QSUB, :],
                                  o_sb[:, h:, :])
            else:
                nc.sync.dma_start(out_r[:, c * QSUB:(c + 1) * QSUB, :],
                                  o_sb[:])

        def flush_oproj(split_dma=False):
            while pending_oproj:
                emit_oproj(*pending_oproj.pop(0), split_dma=split_dma)

        def emit_loads_castT(b, split=False):
            """Load both inputs (kv groups first), cast to bf16 on GPSIMD and
            XBAR-transpose each 4-tile group as soon as it lands.  With
            split=True, only wave 1 (first half of kv and q) is emitted; a
            closure emitting wave 2 is returned so batch 0 can start its
            first projections between the waves."""
            xq_r = xq_d[b].rearrange("(t p) d -> p t d", p=P)
            xkv_r = xkv_d[b].rearrange("(t p) d -> p t d", p=P)
            xq_sb = xin_pool.tile([P, NT_Q, D], f32, tag="xin")
            xkv_sb = xin_pool.tile([P, NT_KV, D], f32, tag="xin")
            xq_bf = xbf_pool.tile([P, NT_Q, D], bf16, tag="xbf")
            xkv_bf = xbf_pool.tile([P, NT_KV, D], bf16, tag="xbf")
            xqT = xt_pool.tile([P, NT_Q, P], bf16, tag="xt")
            xkvT = xt_pool.tile([P, NT_KV, P], bf16, tag="xt")
            wave1 = [(xq_r, xq_sb, xq_bf, xqT, 0),
                     (xkv_r, xkv_sb, xkv_bf, xkvT, 0),
                     (xkv_r, xkv_sb, xkv_bf, xkvT, 4),
                     (xq_r, xq_sb, xq_bf, xqT, 4)]
            wave2 = [(xkv_r, xkv_sb, xkv_bf, xkvT, 8),
                     (xkv_r, xkv_sb, xkv_bf, xkvT, 12),
                     (xq_r, xq_sb, xq_bf, xqT, 8),
                     (xq_r, xq_sb, xq_bf, xqT, 12)]

            def emit_wave(wave, interleave=False):
                if interleave:
                    # batch-0 wave 1: slot each group's transpose into the SP
                    # queue right after the NEXT group's load, so the first
                    # transpose grabs the DMA token as soon as its cast lands
                    prev = None
                    for src, sb, bf, xt, g in wave:
                        nc.sync.dma_start(sb[:, g:g + 4, :],
                                          src[:, g:g + 4, :])
                        nc.gpsimd.tensor_copy(bf[:, g:g + 4, :],
                                              sb[:, g:g + 4, :])
                        if prev is not None:
                            pbf, pxt, pg = prev
                            nc.sync.dma_start_transpose(
                                pxt[:, pg:pg + 4, :],
                                pbf[:, pg:pg + 4, :].rearrange(
                                    "p t d -> p (t d)"))
                        prev = (bf, xt, g)
                    pbf, pxt, pg = prev
                    nc.sync.dma_start_transpose(
                        pxt[:, pg:pg + 4, :],
                        pbf[:, pg:pg + 4, :].rearrange("p t d -> p (t d)"))
                    return
                for src, sb, bf, xt, g in wave:
                    nc.sync.dma_start(sb[:, g:g + 4, :], src[:, g:g + 4, :])
                for src, sb, bf, xt, g in wave:
                    nc.gpsimd.tensor_copy(bf[:, g:g + 4, :],
                                          sb[:, g:g + 4, :])
                    nc.sync.dma_start_transpose(
                        xt[:, g:g + 4, :],
                        bf[:, g:g + 4, :].rearrange("p t d -> p (t d)"))

            emit_wave(wave1, interleave=split)
            if split:
                return xqT, xkvT, lambda: emit_wave(wave2)
            emit_wave(wave2)
            return xqT, xkvT

        def make_proj_units(xqT, xkvT):
            """KT/QT/V projections for one batch as 6 deferrable units that
            get slotted into the previous batch's chunk pipelines."""
            KT = qkv_pool.tile([P, SKV], bf16, tag="KT")
            QT = qkv_pool.tile([P, SQ], bf16, tag="QT")
            V = qkv_pool.tile([P, NT_KV, P], bf16, tag="V")

            def kq_half(dst, w, xt, bias, half):
                def emit():
                    ps = e_psum.tile([P, 2 * QCHUNK], f32, tag="e")
                    for q2 in range(2):
                        g = (half * 2 + q2) * 4
                        nc.tensor.matmul(
                            ps[:, q2 * QCHUNK:(q2 + 1) * QCHUNK],
                            lhsT=wT[w][:], rhs=xt[:, g:g + 4, :],
                            start=True, stop=True)
                    nc.vector.tensor_scalar_add(
                        dst[:, half * 1024:(half + 1) * 1024], ps[:],
                        bcol[bias])
                return emit

            def v_half(half):
                def emit():
                    ps = e_psum.tile([P, 2 * QCHUNK], f32, tag="e")
                    for j in range(8):
                        k = half * 8 + j
                        nc.tensor.matmul(ps[:, j * P:(j + 1) * P],
                                         lhsT=xkvT[:, k, :], rhs=wT["wv"][:],
                                         start=True, stop=True)
                    for q2 in range(2):
                        nc.vector.tensor_copy(
                            V[:, half * 8 + q2 * 4:half * 8 + (q2 + 1) * 4, :],
                            ps[:, q2 * QCHUNK:(q2 + 1) * QCHUNK])
                return emit

            units = [kq_half(KT, "wk", xkvT, "bk", 0),
                     kq_half(QT, "wq", xqT, "bq", 0),
                     v_half(0),
                     kq_half(KT, "wk", xkvT, "bk", 1),
                     v_half(1),
                     kq_half(QT, "wq", xqT, "bq", 1)]
            return (KT, QT, V), units

        def make_proj_units_b0(xqT, xkvT):
            """Batch 0 only: quarter-granularity projections so the first
            E-matmul waits on just one transpose group, minimizing the
            startup ACT gap.  Returns 3 head units (emitted inline) + 6
            deferred units for chunk 0's early slots."""
            KT = qkv_pool.tile([P, SKV], bf16, tag="KT")
            QT = qkv_pool.tile([P, SQ], bf16, tag="QT")
            V = qkv_pool.tile([P, NT_KV, P], bf16, tag="V")

            def kq_q(dst, w, xt, bias, g, on_act=False):
                def emit():
                    ps = e_psum.tile([P, 2 * QCHUNK], f32, tag="e")
                    nc.tensor.matmul(ps[:, :QCHUNK], lhsT=wT[w][:],
                                     rhs=xt[:, 4 * g:4 * g + 4, :],
                                     start=True, stop=True)
                    if on_act:
                        # prologue only: ACT is idle until the first exp and
                        # this bias gates E0 anyway; keeps the DVE chain off
                        # the startup critical path
                        nc.scalar.add(
                            dst[:, g * QCHUNK:(g + 1) * QCHUNK],
                            ps[:, :QCHUNK], bcol[bias])
                    else:
                        nc.vector.tensor_scalar_add(
                            dst[:, g * QCHUNK:(g + 1) * QCHUNK],
                            ps[:, :QCHUNK], bcol[bias])
                return emit

            def kq_h1(dst, w, xt, bias):
                def emit():
                    ps = e_psum.tile([P, 2 * QCHUNK], f32, tag="e")
                    for q2 in range(2):
                        nc.tensor.matmul(
                            ps[:, q2 * QCHUNK:(q2 + 1) * QCHUNK],
                            lhsT=wT[w][:],
                            rhs=xt[:, (2 + q2) * 4:(3 + q2) * 4, :],
                            start=True, stop=True)
                    nc.vector.tensor_scalar_add(
                        dst[:, 1024:2048], ps[:], bcol[bias])
                return emit

            def v_q(g):
                def emit():
                    ps = e_psum.tile([P, 2 * QCHUNK], f32, tag="e")
                    for j in range(4):
                        k = 4 * g + j
                        nc.tensor.matmul(ps[:, j * P:(j + 1) * P],
                                         lhsT=xkvT[:, k, :], rhs=wT["wv"][:],
                                         start=True, stop=True)
                    nc.vector.tensor_copy(
                        V[:, 4 * g:4 * (g + 1), :], ps[:, :QCHUNK])
                return emit

            def v_h1():
                def emit():
                    ps = e_psum.tile([P, 2 * QCHUNK], f32, tag="e")
                    for j in range(8):
                        k = 8 + j
                        nc.tensor.matmul(ps[:, j * P:(j + 1) * P],
                                         lhsT=xkvT[:, k, :], rhs=wT["wv"][:],
                                         start=True, stop=True)
                    for q2 in range(2):
                        nc.vector.tensor_copy(
                            V[:, 8 + q2 * 4:8 + (q2 + 1) * 4, :],
                            ps[:, q2 * QCHUNK:(q2 + 1) * QCHUNK])
                return emit

            head = [kq_q(QT, "wq", xqT, "bq", 0, on_act=True),
                    kq_q(KT, "wk", xkvT, "bk", 0, on_act=True),
                    v_q(0)]
            rest = [kq_q(KT, "wk", xkvT, "bk", 1),
                    v_q(1),
                    kq_h1(KT, "wk", xkvT, "bk"),
                    kq_q(QT, "wq", xqT, "bq", 1),
                    v_h1(),
                    kq_h1(QT, "wq", xqT, "bq")]
            return (KT, QT, V), head, rest

        batches = [bb for _ in range(n_iters) for bb in range(n_batch)]

        # prologue: batch 0 wave-1 loads, then its first-half projections so
        # chunk 0 can start; wave 2 + wd/bd consts follow, and the
        # second-half projections are slotted into chunk 0's pipeline
        xqT0, xkvT0, emit_wave2 = emit_loads_castT(batches[0], split=True)
        cur_qkv, head0, rest0 = make_proj_units_b0(xqT0, xkvT0)
        head0[0]()  # KT quarter 0
        head0[1]()  # QT quarter 0
        head0[2]()  # V quarter 0
        emit_wave2()
        pending_proj = (list(rest0[:3]) + [emit_consts_late]
                        + list(rest0[3:]))
        b0_fill = True

        for bi, b in enumerate(batches):
            KT, QT, V = cur_qkv

            # prefetch + pre-transpose next batch's inputs; its projections
            # become deferred units slotted into this batch's chunk pipelines
            if bi + 1 < len(batches):
                nxqT, nxkvT = emit_loads_castT(batches[bi + 1])
                cur_qkv, nunits = make_proj_units(nxqT, nxkvT)
                pending_proj = pending_proj + list(nunits)

            # attention, skv-major supertiles, software-pipelined
            FT = ft_pool.tile([P, SQ], bf16, tag="FT")
            recipS = r_pool.tile([P, NT_Q], f32, tag="r")
            for c in range(NC_Q):
                qsl = slice(c * QCHUNK, (c + 1) * QCHUNK)
                f_ps = f_psum.tile([P, QCHUNK], f32, tag="f")
                a_tiles = [None] * NST
                l1 = [None] * NST
                l2 = [None] * (NST // 2)
                l3 = [None] * (NST // 4)
                for st in range(NST + LA):
                    if st < NST:
                        e_ps = e_psum.tile([P, 2 * QCHUNK], f32, tag="e")
                        for h in range(2):
                            k = 2 * st + h
                            nc.tensor.matmul(
                                e_ps[:, h * QCHUNK:(h + 1) * QCHUNK],
                                lhsT=KT[:, k * P:(k + 1) * P],
                                rhs=QT[:, qsl], start=True, stop=True)
                        a_sb = a_pool.tile([P, 2 * QCHUNK], bf16, tag="a")
                        nc.scalar.activation(a_sb[:], e_ps[:], AF.Exp,
                                             scale=SCALE)
                        a_tiles[st] = a_sb
                    if st == LA:
                        # slot the previous chunk's O-projection into this
                        # chunk's pipeline so PE never blocks on recipS
                        flush_oproj()
                    if pending_proj and (
                            (b0_fill and c == 0 and st <= 6)
                            or (c >= 1 and st in (4, 6))):
                        # slot deferred KT/QT/V projection units (batch 0's
                        # second half, or the next batch's) into the pipeline
                        # so no PE lump starves the ACT queue
                        pending_proj.pop(0)()
                    if st >= LA:
                        t = st - LA
                        a_sb = a_tiles[t]
                        for h in range(2):
                            k = 2 * t + h
                            nc.tensor.matmul(
                                f_ps[:], lhsT=V[:, k, :],
                                rhs=a_sb[:, h * QCHUNK:(h + 1) * QCHUNK],
                                start=(k == 0), stop=(k == NT_KV - 1))
                        # bf16 pairwise add-tree for the softmax row sums
                        d1 = tr_pool.tile([P, QCHUNK], bf16, tag="t")
                        nc.gpsimd.tensor_add(d1[:], a_sb[:, :QCHUNK],
                                             a_sb[:, QCHUNK:])
                        l1[t] = d1
                        if t % 2 == 1:
                            d2 = tr_pool.tile([P, QCHUNK], bf16, tag="t")
                            nc.vector.tensor_add(d2[:], l1[t - 1][:], l1[t][:])
                            l2[t // 2] = d2
                        if t % 4 == 3:
                            d3 = tr_pool.tile([P, QCHUNK], bf16, tag="t")
                            nc.vector.tensor_add(d3[:], l2[t // 2 - 1][:],
                                                 l2[t // 2][:])
                            l3[t // 4] = d3
                if b0_fill and c == 0:
                    b0_fill = False
                # FT copy first: it gates the O-projection matmuls.  On the
                # very last chunk run it on the now-idle ACT so it overlaps
                # the DVE recip chain on the drain path.
                last = (bi == len(batches) - 1 and c == NC_Q - 1)
                if last:
                    nc.scalar.copy(FT[:, qsl], f_ps[:])
                else:
                    nc.vector.tensor_copy(FT[:, qsl], f_ps[:])
                # denominators, already transposed: S^T[q, 1] per subtile,
                # accumulated from the two l3 halves (skips the final tree
                # add on the drain path); per-subtile reciprocal so the
                # O-projection chain pipelines on the last chunk
                st_ps = m_psum.tile([P, QSUB], f32, tag="m")
                # on the drain path, sum straight from the 8 L1 partials so
                # recipS does not wait the L2/L3 reduction chain
                srcs = l1 if last else l3
                for j in range(QSUB):
                    for hv, lt in enumerate(srcs):
                        nc.tensor.matmul(st_ps[:, j:j + 1],
                                         lhsT=lt[:, j * P:(j + 1) * P],
                                         rhs=ones_col[:], start=(hv == 0),
                                         stop=(hv == len(srcs) - 1))
                    nc.vector.reciprocal(
                        recipS[:, c * QSUB + j:c * QSUB + j + 1],
                        st_ps[:, j:j + 1])
                pending_oproj.append((FT, recipS, b, c))

            while pending_proj:
                pending_proj.pop(0)()

        flush_oproj(split_dma=True)

    nc.compile()
    return nc


def get_program(n_batch=B_PER_CORE, n_iters=1):
    key = (n_batch, n_iters)
    if key not in _PROGRAM_CACHE:
        _PROGRAM_CACHE[key] = build_program(n_batch, n_iters)
    return _PROGRAM_CACHE[key]


def pack_wb(Wv, bv, Wk, bk, Wq, bq, Wd, bd):
    """Pack all params into the single [128, 516] const tensor the program
    loads with one DMA: wk|wq|wv|wd natural, then bk|bq|bv|bd columns.

    The V bias is folded host-side: A @ (V0 + ones bv^T) = F0 + S bv^T, and
    after the 1/S softmax normalization that term is the constant row
    bv^T Wd^T, so bd' = bd + Wd @ bv and the device applies no bv at all."""
    bd_folded = (np.asarray(bd, np.float64)
                 + np.asarray(Wd, np.float64) @ np.asarray(bv, np.float64))
    return np.ascontiguousarray(np.concatenate(
        [np.asarray(Wk, np.float32), np.asarray(Wq, np.float32),
         np.asarray(Wv, np.float32), np.asarray(Wd, np.float32),
         np.asarray(bk, np.float32)[:, None],
         np.asarray(bq, np.float32)[:, None],
         np.zeros((128, 1), np.float32),
         bd_folded.astype(np.float32)[:, None]], axis=1))


def kernel(smiles_features, image_features, Wv, bv, Wk, bk, Wq, bq, Wd, bd,
           _trace=False):
    from concourse.bass_utils import run_bass_kernel_spmd

    smiles_features = np.ascontiguousarray(smiles_features, dtype=np.float32)
    image_features = np.ascontiguousarray(image_features, dtype=np.float32)
    consts = {"wb": pack_wb(Wv, bv, Wk, bk, Wq, bq, Wd, bd)}

    nc = get_program()
    in_maps = []
    for core in range(N_CORES):
        lo = core * B_PER_CORE
        hi = lo + B_PER_CORE
        m = dict(consts)
        m["xq"] = image_features[lo:hi]
        m["xkv"] = smiles_features[lo:hi]
        in_maps.append(m)

    res = run_bass_kernel_spmd(nc, in_maps, list(range(N_CORES)),
                               trace=_trace)
    out = np.concatenate([r["out"] for r in res.results], axis=0)
    if _trace:
        return out, res
    return out
